# revision 25
# baseline (speedup 1.0000x reference)
"""Trainium2 Bass kernel for nn_MoEAD_43611097924200 (MoE transformer decoder).

8 NeuronCores; core c owns batch b=c//2, sequence half c%2 (512 tokens).
Feature-major activations [128, 2, cols]; token-major [128, 4, 256].
One pairwise AllGather exchanges `dec` between batch halves after layer 0.
MHA#1 is layer-invariant in the reference -> computed once.
MoE: top-2/8 routed, capacity 192/expert/core, indirect-DMA dispatch+combine.
"""
import math
import sys

sys.path.insert(0, "/opt/trn_rl_repo")

import numpy as np
import ml_dtypes

import concourse.bass as bass
import concourse.mybir as mybir
import concourse.tile as tile
from concourse.vector_clock import ScopedClock
from concourse.masks import make_identity, make_upper_triangular
from concourse.bass import IndirectOffsetOnAxis

# ---------------------------------------------------------------------------
# Walrus workaround: this toolchain allows only ONE sem wait per instruction.
# ---------------------------------------------------------------------------
MAX_WAITS = 1
_ws_counter = [0]
_orig_postorder = tile.postorder_instruction_blocks


def _split_inst_waits(insts):
    out = []
    for inst in insts:
        si = getattr(inst, "sync_info", None)
        waits = list(si.on_wait) if si is not None else []
        if len(waits) > MAX_WAITS:
            for i in range(0, len(waits) - MAX_WAITS, MAX_WAITS):
                _ws_counter[0] += 1
                out.append(mybir.InstNoOp(
                    name=f"I-waitsplit-{_ws_counter[0]}",
                    sync_info=mybir.SyncInfo(on_wait=waits[i:i + MAX_WAITS], on_update=[]),
                    engine=inst.engine, bass_nofuse=True))
            inst.sync_info = mybir.SyncInfo(
                on_wait=waits[len(waits) - MAX_WAITS:], on_update=list(si.on_update))
        out.append(inst)
    return out


def _patched_postorder(instructions, start_bb, output):
    if not output:
        for bb_name in list(instructions.keys()):
            instructions[bb_name] = _split_inst_waits(instructions[bb_name])
    return _orig_postorder(instructions, start_bb, output)


def _patched_drain_and_barrier(self, tick_clock, wait_clock):
    nc = self.nc
    probe = nc.sync.nop(nofuse=True, hint="drain_wait_split")
    wait_clock.add_sem_waits(probe.ins, ScopedClock({None: tick_clock.global_clock}))
    si = probe.ins.sync_info
    waits = list(si.on_wait) if si is not None else []
    probe.ins.sync_info = mybir.SyncInfo(on_wait=waits[:MAX_WAITS], on_update=[])
    for i in range(MAX_WAITS, len(waits), MAX_WAITS):
        extra = nc.sync.nop(nofuse=True, hint="drain_wait_split")
        extra.ins.sync_info = mybir.SyncInfo(on_wait=waits[i:i + MAX_WAITS], on_update=[])
    nc.sync.drain()
    nc.all_engine_barrier()
    assert self.sems is not None
    popped = nc._tile_sem_poison_stack.pop()
    assert popped is self._sem_poison
    nc.clear_and_free_semaphores(list(self.sems.allocated().values()))
    nc.all_engine_barrier()


tile.TileContext._drain_and_barrier = _patched_drain_and_barrier
tile.postorder_instruction_blocks = _patched_postorder

# ---------------------------------------------------------------------------
B, CIN, H, W = 4, 272, 32, 32
D, NHEAD, DFF, E, TOPK, NDEC = 256, 8, 1024, 8, 2, 2
HW = H * W
DH = D // NHEAD
N_CORES = 8
HALF = 512               # tokens per core
CAP = 192                # expert capacity per (core, expert)
NSLOT = E * CAP
CINP = 384               # CIN padded to 3*128
F32 = mybir.dt.float32
F32R = mybir.dt.float32r
BF16 = mybir.dt.bfloat16
I32 = mybir.dt.int32
AF = mybir.ActivationFunctionType
ALU = mybir.AluOpType
SCALE_QK = 1.0 / math.sqrt(DH)


def _sine_pos():
    npf = D // 2
    ye = np.cumsum(np.ones((H, W), np.float32), 0)
    xe = np.cumsum(np.ones((H, W), np.float32), 1)
    eps, scale = 1e-6, 2.0 * np.pi
    ye = ye / (ye[-1:, :] + eps) * scale
    xe = xe / (xe[:, -1:] + eps) * scale
    t = np.arange(npf, dtype=np.float32)
    dim_t = (10000.0 ** (2.0 * np.floor(t / 2.0) / npf)).astype(np.float32)
    px = xe[:, :, None] / dim_t
    py = ye[:, :, None] / dim_t
    px = np.stack((np.sin(px[:, :, 0::2]), np.cos(px[:, :, 1::2])), axis=3).reshape(H, W, npf)
    py = np.stack((np.sin(py[:, :, 0::2]), np.cos(py[:, :, 1::2])), axis=3).reshape(H, W, npf)
    return np.concatenate((py, px), axis=2).reshape(HW, D).astype(np.float32)


def r32(ap):
    # fp32r requires verifier-visible rounding of every producer; use fp32.
    return ap


def build_nc():
    nc = bass.Bass()
    names = {}

    def din(name, shape, dtype=F32):
        t = nc.dram_tensor(name, list(shape), dtype, kind="ExternalInput")
        names[name] = t
        return t

    dd = {}
    dd["xT"] = din("xT", [CINP, HW], F32R)                 # own batch, feature-major, CIN-padded
    dd["posT"] = din("posT", [D, HW], F32R)
    dd["posTh"] = din("posTh", [D, HALF])            # own half of pos
    dd["leT"] = din("leT", [D, HALF])                # own half of learned_embed.T
    dd["WiT"] = din("WiT", [CINP, D], F32R)
    dd["bi"] = din("bi", [1, D], F32R)
    for t in ("s", "c"):
        for w in ("q", "k", "v", "o"):
            dd[f"W{w}{t}T"] = din(f"W{w}{t}T", [D, D], F32R)
            dd[f"b{w}{t}"] = din(f"b{w}{t}", [1, D], F32R)
    dd["ln_gf"] = din("ln_gf", [1, 4 * D]); dd["ln_bf"] = din("ln_bf", [1, 4 * D])
    dd["ln_gt"] = din("ln_gt", [1, 4 * D]); dd["ln_bt"] = din("ln_bt", [1, 4 * D])
    dd["gw1T"] = din("gw1T", [D, E]); dd["gb1"] = din("gb1", [1, E])
    dd["gw2T"] = din("gw2T", [D, E]); dd["gb2"] = din("gb2", [1, E])
    dd["W1a"] = din("W1a", [E, D, DFF], BF16); dd["W2a"] = din("W2a", [E, DFF, D], BF16)
    dd["W1b"] = din("W1b", [E, D, DFF], BF16); dd["W2b"] = din("W2b", [E, DFF, D], BF16)
    dd["b1a"] = din("b1a", [E, DFF]); dd["b2a"] = din("b2a", [E, D])
    dd["b1b"] = din("b1b", [E, DFF]); dd["b2b"] = din("b2b", [E, D])
    dd["WoutT"] = din("WoutT", [D, CINP], F32R)
    dd["bout"] = din("bout", [1, CINP], F32R)
    dd["capbase"] = din("capbase", [1, E])
    dd["onesr"] = din("onesr", [1, 512], F32R)
    dd["agrows"] = din("agrows", [128, 4], I32)

    out_d = nc.dram_tensor("recT", [CINP, HALF], F32, kind="ExternalOutput")
    xg_dram = nc.dram_tensor("xg_dram", [NSLOT, D], BF16)
    y_dram = nc.dram_tensor("y_dram", [NSLOT, D], BF16)
    cc_in = nc.dram_tensor("cc_in", [D, HALF], F32R)
    cc_out = nc.dram_tensor("cc_out", [N_CORES * D, HALF], F32R, addr_space="Shared")

    with tile.TileContext(nc) as tc:
        _body(nc, tc, dd, out_d, xg_dram, y_dram, cc_in, cc_out)
    return nc


def _r3(dram2d, p=128):
    """[A*p, N] dram -> [p, A, N] view"""
    return dram2d[:, :].rearrange("(a p) n -> p a n", p=p)


def _body(nc, tc, dd, out_d, xg_dram, y_dram, cc_in, cc_out):
    from contextlib import ExitStack
    with ExitStack() as ctx:
        const = ctx.enter_context(tc.tile_pool(name="const", bufs=1))
        persist = ctx.enter_context(tc.tile_pool(name="persist", bufs=1))
        work = ctx.enter_context(tc.tile_pool(name="work", bufs=1))
        small = ctx.enter_context(tc.tile_pool(name="small", bufs=2))
        ps = ctx.enter_context(tc.tile_pool(name="ps", bufs=2, space="PSUM"))
        wpool = ctx.enter_context(tc.tile_pool(name="wpool", bufs=1))

        _gpn = [0]

        def gp(shape, dtype=F32):
            _gpn[0] += 1
            return ps.tile(shape, dtype, tag="gp", name=f"gp{_gpn[0]}")

        # ---------------- constants ----------------
        ident = const.tile([128, 128], F32); make_identity(nc, ident[:])
        ident_bf = const.tile([128, 128], BF16)
        nc.vector.tensor_copy(ident_bf[:], ident[:])
        utri = const.tile([128, 128], F32)
        make_upper_triangular(nc, utri[:], val=1.0, diag=False)   # 1 iff row<col
        ones2d = const.tile([128, 128], F32); nc.vector.memset(ones2d[:], 1.0)
        onesrow = const.tile([1, 512], F32R)
        nc.sync.dma_start(out=onesrow[:], in_=dd["onesr"][:, :])
        onesf = const.tile([1, 512], F32); nc.vector.memset(onesf[:], 1.0)
        eps128 = const.tile([128, 1], F32); nc.vector.memset(eps128[:], 1e-5)

        def loadc(name, shape3=None, dtype=None, pool=const):
            dr = dd[name]
            if shape3 is None:
                t = pool.tile(list(dr.shape), dtype or dr.dtype,
                              tag="c_" + name, name="c_" + name)
                nc.sync.dma_start(out=t[:], in_=dr[:, :])
            else:
                t = pool.tile(shape3, dtype or dr.dtype,
                              tag="c_" + name, name="c_" + name)
                nc.sync.dma_start(out=t[:], in_=_r3(dr))
            return t

        agrows = const.tile([128, 4], I32)
        nc.sync.dma_start(out=agrows[:], in_=dd["agrows"][:, :])
        capbase = const.tile([128, E], F32)
        nc.sync.dma_start(out=capbase[:], in_=dd["capbase"][:, :].to_broadcast([128, E]))
        wiT = work.tile([128, 3, D], F32R, tag="kin", name="wiT_t")
        nc.sync.dma_start(out=wiT[:], in_=_r3(dd["WiT"]))
        bi = loadc("bi")
        wat = {}
        for t in ("s", "c"):
            for w in ("q", "k", "v", "o"):
                wat[w + t] = loadc(f"W{w}{t}T", [128, 2, D])
                wat["b" + w + t] = loadc(f"b{w}{t}")
        lngf = loadc("ln_gf"); lnbf = loadc("ln_bf")
        lngt = work.tile([1, 4 * D], F32, tag="expsb", name="lngt_t")
        nc.sync.dma_start(out=lngt[:], in_=dd["ln_gt"][:, :])
        lnbt = work.tile([1, 4 * D], F32, tag="moexg", name="lnbt_t")
        nc.sync.dma_start(out=lnbt[:], in_=dd["ln_bt"][:, :])
        gw1 = loadc("gw1T", [128, 2, E]); gb1 = loadc("gb1")
        gw2 = loadc("gw2T", [128, 2, E]); gb2 = loadc("gb2")
        # expert biases as per-partition columns: [E, DFF] -> [128, E, 8]
        b1sb, b2sb = {}, {}
        for key in ("a", "b"):
            t = const.tile([128, E, DFF // 128], F32, tag="b1c" + key, name="b1c" + key)
            nc.sync.dma_start(out=t[:], in_=dd["b1" + key][:, :].rearrange("e (f p) -> p e f", p=128))
            b1sb[key] = t
            t2 = const.tile([128, E, 2], F32, tag="b2c" + key, name="b2c" + key)
            nc.sync.dma_start(out=t2[:], in_=dd["b2" + key][:, :].rearrange("e (m p) -> p e m", p=128))
            b2sb[key] = t2
        bout = loadc("bout")
        posT = loadc("posT", [128, 2, HW])
        posTh = loadc("posTh", [128, 2, HALF])
        leT = loadc("leT", [128, 2, HALF])
        xT = work.tile([128, 3, HW], F32R, tag="bigx", name="xT_t")
        nc.sync.dma_start(out=xT[:], in_=_r3(dd["xT"]))

        # zero-init xg_dram so padded expert slots read zeros (also keeps sim happy)
        ztile = const.tile([128, 1, D], BF16, tag="ztile", name="ztile")
        nc.vector.memset(ztile[:], 0.0)
        nc.sync.dma_start(out=_r3(xg_dram), in_=ztile[:].to_broadcast([128, NSLOT // 128, D]))

        # token-major LN g/b broadcast tiles for ln sets 2,3,6,7
        gbc = {}
        for fi, il in enumerate((2, 3, 6, 7)):
            pair = []
            for srcrow in (lngt, lnbt):
                pt = gp([128, D])
                for d in range(2):
                    nc.tensor.matmul(out=pt[:, d * 128:(d + 1) * 128],
                                     lhsT=r32(ones2d[0:1, :]),
                                     rhs=r32(srcrow[:, fi * D + d * 128:fi * D + (d + 1) * 128]),
                                     start=True, stop=True)
                t = persist.tile([128, D], F32, tag=f"gbc{il}{len(pair)}", name=f"gbc{il}{len(pair)}")
                nc.vector.tensor_copy(t[:], pt[:])
                pair.append(t)
            gbc[il] = pair

        # ---------------- generic helpers ----------------
        def proj(wT, brow, rhs, ncols, nk, out=None, out_tag="proj", nm=2, mt=128, odt=F32):
            """out [128, nm, ncols] (f32) = wT.T @ rhs + b.
            wT [128, nk, nm*128]; rhs [128, nk, ncols]."""
            if out is None:
                out = work.tile([128, nm, ncols], odt, tag=out_tag, name=out_tag + "_p")
            for m in range(nm):
                mc = mt if m == nm - 1 else 128
                for n0 in range(0, ncols, 512):
                    cw = min(512, ncols - n0)
                    p = gp([128, 512])
                    for k in range(nk):
                        nc.tensor.matmul(out=p[:mc, :cw],
                                         lhsT=r32(wT[:, k, m * 128:m * 128 + mc]),
                                         rhs=r32(rhs[:, k, n0:n0 + cw]),
                                         start=(k == 0), stop=False)
                    nc.tensor.matmul(out=p[:mc, :cw],
                                     lhsT=r32(brow[:, m * 128:m * 128 + mc]),
                                     rhs=r32(onesrow[:, :cw]), start=False, stop=True)
                    nc.vector.tensor_copy(out[:mc, m, n0:n0 + cw], p[:mc, :cw])
            return out

        def v_tokens(featT, wvT, bvrow, tlen):
            """v33 [128, tlen//128, 264] bf16; per head cols 33h..+31 = v_h, col 33h+32 = 1"""
            v33 = work.tile([128, tlen // 128, NHEAD * 33], BF16, tag="v33")
            for tt in range(tlen // 128):
                p = gp([128, D])
                for k in range(2):
                    nc.tensor.matmul(out=p[:],
                                     lhsT=r32(featT[:, k, tt * 128:(tt + 1) * 128]),
                                     rhs=r32(wvT[:, k, :]), start=(k == 0), stop=False)
                nc.tensor.matmul(out=p[:], lhsT=r32(onesrow[:, 0:128].rearrange('a n -> a n')), rhs=r32(bvrow[:, :]),
                                 start=False, stop=True)
                dst = v33[:, tt, :].rearrange("p (h x) -> p h x", h=NHEAD)
                src = p[:].rearrange("p (h x) -> p h x", h=NHEAD)
                nc.vector.tensor_copy(dst[:, :, 0:DH], src[:, :, :])
                nc.vector.memset(dst[:, :, DH:DH + 1], 1.0)
            return v33

        def attn(qT, kT, v33, woT, borow, tlen, out_tag):
            """Returns attnT [128, 2, 512] f32 (after out-proj+bias)."""
            ntt = tlen // 128
            o_sb = work.tile([128, 2, HALF], F32R, tag="osb")
            for g in range(2):
                for hh in range(4):
                    h = g * 4 + hh
                    exp_sb = work.tile([128, ntt, HALF], BF16, tag="expsb")
                    for chunk in range(ntt // 2):
                        sc = gp([128, 1024])
                        for j in range(2):
                            tt = chunk * 2 + j
                            nc.tensor.matmul(
                                out=sc[:, j * HALF:(j + 1) * HALF],
                                lhsT=r32(kT[hh * DH:(hh + 1) * DH, g, tt * 128:(tt + 1) * 128]),
                                rhs=r32(qT[hh * DH:(hh + 1) * DH, g, :]),
                                start=True, stop=True,
                                tile_position=(hh * DH, 0))
                        nc.scalar.activation(
                            out=exp_sb[:, chunk * 2:(chunk + 1) * 2, :].rearrange("p a n -> p (a n)"),
                            in_=sc[:], func=AF.Exp, scale=SCALE_QK)
                    po = ps.tile([33, HALF], F32, tag="avps")
                    for tt in range(ntt):
                        nc.tensor.matmul(out=po[:],
                                         lhsT=v33[:, tt, h * 33:(h + 1) * 33],
                                         rhs=exp_sb[:, tt, :],
                                         start=(tt == 0), stop=(tt == ntt - 1))
                    rec = small.tile([1, HALF], F32, tag="avrec", bufs=1)
                    nc.vector.reciprocal(out=rec[:], in_=po[32:33, :])
                    rb = ps.tile([DH, HALF], F32, tag="avrb")
                    nc.tensor.matmul(out=rb[:], lhsT=r32(ones2d[0:1, :DH]), rhs=r32(rec[:]),
                                     start=True, stop=True)
                    nc.vector.tensor_copy(o_sb[hh * DH:(hh + 1) * DH, g, :], po[0:DH, :])
                    nc.vector.tensor_tensor(out=o_sb[hh * DH:(hh + 1) * DH, g, :],
                                            in0=o_sb[hh * DH:(hh + 1) * DH, g, :],
                                            in1=rb[:], op=ALU.mult)
            return proj(woT, borow, o_sb, HALF, 2, out_tag=out_tag)

        _LNF = {0: 0, 1: 1, 4: 2, 5: 3}

        def ln_feat(in_sb, il, out_tag):
            il = _LNF[il]
            """LN over d; in/out [128, 2, 512] f32."""
            out_sb = work.tile([128, 2, HALF], F32, tag=out_tag, name=out_tag + "_f")
            sq = work.tile([128, 2, HALF], F32, tag="osb", name="lnsq_t")
            for d in range(2):
                nc.vector.tensor_tensor(out=sq[:, d, :], in0=in_sb[:, d, :],
                                        in1=in_sb[:, d, :], op=ALU.mult)
            st = gp([1, HALF])
            for d in range(2):
                nc.tensor.matmul(out=st[:, :], lhsT=r32(ones2d[:, 0:1]),
                                 rhs=r32(in_sb[:, d, :]), start=(d == 0), stop=(d == 1))
            st2 = gp([1, HALF])
            for d in range(2):
                nc.tensor.matmul(out=st2[:, :], lhsT=r32(ones2d[:, 0:1]),
                                 rhs=r32(sq[:, d, :]), start=(d == 0), stop=(d == 1))
            mean = small.tile([1, HALF], F32, tag="lnmean", bufs=1)
            nc.vector.tensor_scalar_mul(mean[:], st[:, :], 1.0 / D)
            var = small.tile([1, HALF], F32, tag="lnvar", bufs=1)
            nc.vector.tensor_scalar_mul(var[:], st2[:, :], 1.0 / D)
            m2 = small.tile([1, HALF], F32, tag="lnm2", bufs=1)
            nc.vector.tensor_tensor(out=m2[:], in0=mean[:], in1=mean[:], op=ALU.mult)
            nc.vector.tensor_sub(out=var[:], in0=var[:], in1=m2[:])
            rstd = small.tile([1, HALF], F32, tag="lnrstd", bufs=1)
            nc.scalar.activation(out=rstd[:], in_=var[:], func=AF.Ln, bias=eps128[0:1, :], scale=1.0)
            nc.scalar.activation(out=rstd[:], in_=rstd[:], func=AF.Exp, scale=-0.5)
            nmr = small.tile([1, HALF], F32, tag="lnnmr", bufs=1)
            nc.vector.tensor_tensor(out=nmr[:], in0=mean[:], in1=rstd[:], op=ALU.mult)
            nc.vector.tensor_scalar_mul(nmr[:], nmr[:], -1.0)
            for d in range(2):
                G = gp([128, HALF])
                nc.tensor.matmul(out=G[:], lhsT=r32(lngf[:, il * D + d * 128:il * D + (d + 1) * 128]),
                                 rhs=r32(rstd[:]), start=True, stop=True)
                Hp = gp([128, HALF])
                nc.tensor.matmul(out=Hp[:], lhsT=r32(lngf[:, il * D + d * 128:il * D + (d + 1) * 128]),
                                 rhs=r32(nmr[:]), start=True, stop=False)
                nc.tensor.matmul(out=Hp[:], lhsT=r32(lnbf[:, il * D + d * 128:il * D + (d + 1) * 128]),
                                 rhs=r32(onesf[:, :HALF]), start=False, stop=True)
                nc.vector.tensor_tensor(out=out_sb[:, d, :], in0=in_sb[:, d, :], in1=G[:], op=ALU.mult)
                nc.vector.tensor_tensor(out=out_sb[:, d, :], in0=out_sb[:, d, :], in1=Hp[:], op=ALU.add)
            return out_sb

        def ln_tok(in_sb, il, out_tag):
            """LN over free dim; in/out [128, 4, 256] f32 token-major."""
            out_sb = work.tile([128, 4, D], F32, tag=out_tag, name=out_tag + "_l")
            g128, b128 = gbc[il]
            for t in range(4):
                stats = small.tile([128, 6], F32, tag="bnst")
                nc.vector.bn_stats(out=stats[:], in_=in_sb[:, t, :])
                mv = small.tile([128, 2], F32, tag="bnmv")
                nc.vector.bn_aggr(out=mv[:], in_=stats[:])
                rstd = small.tile([128, 1], F32, tag="bnr")
                nc.scalar.activation(out=rstd[:], in_=mv[:, 1:2], func=AF.Ln, bias=eps128[:, :], scale=1.0)
                nc.scalar.activation(out=rstd[:], in_=rstd[:], func=AF.Exp, scale=-0.5)
                nc.vector.tensor_scalar(out=out_sb[:, t, :], in0=in_sb[:, t, :],
                                        scalar1=mv[:, 0:1], scalar2=rstd[:],
                                        op0=ALU.subtract, op1=ALU.mult)
                nc.vector.tensor_tensor(out=out_sb[:, t, :], in0=out_sb[:, t, :],
                                        in1=g128[:], op=ALU.mult)
                nc.vector.tensor_tensor(out=out_sb[:, t, :], in0=out_sb[:, t, :],
                                        in1=b128[:], op=ALU.add)
            return out_sb

        def feat_to_tok(featT, bf_tag=None, f32_tag=None):
            """[128, 2, 512] feat-major -> token-major [128, 4, 256] (bf16 and/or f32)."""
            tok_bf = (work.tile([128, 4, D], BF16, tag=bf_tag, name=bf_tag + "_t")
                      if bf_tag else None)
            tok_f32 = (work.tile([128, 4, D], F32, tag=f32_tag, name=f32_tag + "_t")
                       if f32_tag else None)
            for t in range(4):
                p = gp([128, D])
                for d in range(2):
                    nc.tensor.transpose(out=p[:, d * 128:(d + 1) * 128],
                                        in_=featT[:, d, t * 128:(t + 1) * 128],
                                        identity=ident[:])
                if tok_bf is not None:
                    nc.vector.tensor_copy(tok_bf[:, t, :], p[:])
                if tok_f32 is not None:
                    nc.vector.tensor_copy(tok_f32[:, t, :], p[:])
            return tok_bf, tok_f32

        def tok_to_feat(tok_sb, out_tag, odt=F32):
            """token-major [128, 4, 256] f32 -> feature-major [128, 2, 512] f32."""
            featT = work.tile([128, 2, HALF], odt, tag=out_tag, name=out_tag + "_tf")
            for d in range(2):
                p = gp([128, HALF])
                for t in range(4):
                    nc.tensor.transpose(out=p[:, t * 128:(t + 1) * 128],
                                        in_=tok_sb[:, t, d * 128:(d + 1) * 128],
                                        identity=ident[:])
                nc.vector.tensor_copy(featT[:, d, :], p[:])
            return featT

        def moe(xT_feat, gwT, gbrow, w1_dram, w2_dram, b1col, b2col, out_tag):
            """x + MoE(x); returns token-major [128, 4, 256] f32."""
            xtok_bf, xtok_f32 = feat_to_tok(xT_feat, bf_tag="moextb", f32_tag="moextf")
            # gate logits (token-major), fp32 exact
            slot_i = []
            s1l, s2l = [], []
            masks = work.tile([128, 4, E], F32, tag="moemask")
            for t in range(4):
                lp = gp([128, E])
                for k in range(2):
                    nc.tensor.matmul(out=lp[:], lhsT=xT_feat[:, k, t * 128:(t + 1) * 128],
                                     rhs=gwT[:, k, :], start=(k == 0), stop=False)
                nc.tensor.matmul(out=lp[:], lhsT=ones2d[0:1, :], rhs=gbrow[:, :],
                                 start=False, stop=True)
                logit = small.tile([128, E], F32, tag="moelog")
                nc.vector.tensor_copy(logit[:], lp[:])
                vals = small.tile([128, 8], F32, tag="moev")
                nc.vector.max(out=vals[:], in_=logit[:])
                m1 = small.tile([128, E], F32, tag="moem1")
                nc.vector.tensor_tensor(out=m1[:], in0=logit[:],
                                        in1=vals[:, 0:1].to_broadcast([128, E]), op=ALU.is_equal)
                m2 = small.tile([128, E], F32, tag="moem2")
                nc.vector.tensor_tensor(out=m2[:], in0=logit[:],
                                        in1=vals[:, 1:2].to_broadcast([128, E]), op=ALU.is_equal)
                nc.vector.tensor_add(out=masks[:, t, :], in0=m1[:], in1=m2[:])
                # gate scores: s1 = 1/(1+exp(v2-v1)), s2 = 1-s1
                dv = small.tile([128, 1], F32, tag="moedv")
                nc.vector.tensor_sub(out=dv[:], in0=vals[:, 1:2], in1=vals[:, 0:1])
                ev = small.tile([128, 1], F32, tag="moeev")
                nc.scalar.activation(out=ev[:], in_=dv[:], func=AF.Exp)
                nc.vector.tensor_scalar_add(ev[:], ev[:], 1.0)
                s1 = small.tile([128, 1], F32, tag="moes1", bufs=5)
                nc.vector.reciprocal(out=s1[:], in_=ev[:])
                s2 = small.tile([128, 1], F32, tag="moes2", bufs=5)
                nc.vector.tensor_sub(out=s2[:], in0=ones2d[:, 0:1], in1=s1[:])
                s1l.append(s1); s2l.append(s2)
                # exclusive cumulative count over tokens (prefix matmuls)
                pp = gp([128, E])
                for k in range(t + 1):
                    nc.tensor.matmul(out=pp[:],
                                     lhsT=(utri[:] if k == t else ones2d[:]),
                                     rhs=masks[:, k, :], start=(k == 0), stop=(k == t))
                slotm = small.tile([128, E], F32, tag="moeslotm")
                nc.vector.tensor_add(out=slotm[:], in0=pp[:], in1=capbase[:])
                for (m, lst) in ((m1, 0), (m2, 1)):
                    tmp = small.tile([128, E], F32, tag="moetmp")
                    nc.vector.tensor_tensor(out=tmp[:], in0=m[:], in1=slotm[:], op=ALU.mult)
                    sf = small.tile([128, 1], F32, tag="moesf")
                    nc.vector.tensor_reduce(out=sf[:], in_=tmp[:], axis=mybir.AxisListType.X,
                                            op=ALU.add)
                    nc.vector.tensor_scalar_min(sf[:], sf[:], float(NSLOT - 1))
                    si = small.tile([128, 1], I32, tag="moesi", bufs=9)
                    nc.vector.tensor_copy(si[:], sf[:])
                    slot_i.append(si)   # order: t0r1, t0r2, t1r1, t1r2, ...
            # dispatch: scatter token rows into expert slots
            for t in range(4):
                for r in range(2):
                    nc.gpsimd.indirect_dma_start(
                        out=xg_dram[:, :],
                        out_offset=IndirectOffsetOnAxis(ap=slot_i[2 * t + r][:, 0:1], axis=0),
                        in_=xtok_bf[:, t, :], in_offset=None)
            xg_sb = work.tile([128, NSLOT // 128, D], BF16, tag="moexg")
            nc.sync.dma_start(out=xg_sb[:], in_=_r3(xg_dram))
            xgT = work.tile([128, 2, NSLOT], BF16, tag="moexgT")
            for c4 in range(NSLOT // 512):
                for d in range(2):
                    p = gp([128, 512], BF16)
                    for j in range(4):
                        nc.tensor.transpose(out=p[:, j * 128:(j + 1) * 128],
                                            in_=xg_sb[:, c4 * 4 + j, d * 128:(d + 1) * 128],
                                            identity=ident_bf[:])
                    nc.vector.tensor_copy(xgT[:, d, c4 * 512:(c4 + 1) * 512], p[:])
            # experts
            for e in range(E):
                w1t = wpool.tile([128, 2, DFF], BF16, tag="w1t")
                nc.sync.dma_start(out=w1t[:], in_=w1_dram[e].rearrange("(k p) f -> p k f", p=128))
                w2t = wpool.tile([128, 8, D], BF16, tag="w2t")
                nc.sync.dma_start(out=w2t[:], in_=w2_dram[e].rearrange("(k p) m -> p k m", p=128))
                hT = work.tile([128, 8, CAP], BF16, tag="moehT")
                for f2 in range(4):
                    hp = gp([128, 2 * CAP])
                    for j in range(2):
                        f = f2 * 2 + j
                        for k in range(2):
                            nc.tensor.matmul(
                                out=hp[:, j * CAP:(j + 1) * CAP],
                                lhsT=w1t[:, k, f * 128:(f + 1) * 128],
                                rhs=xgT[:, k, e * CAP:(e + 1) * CAP],
                                start=(k == 0), stop=(k == 1))
                    for j in range(2):
                        f = f2 * 2 + j
                        nc.vector.tensor_scalar(out=hT[:, f, :], in0=hp[:, j * CAP:(j + 1) * CAP],
                                                scalar1=b1col[:, e, f:f + 1], scalar2=0.0,
                                                op0=ALU.add, op1=ALU.max)
                ytok = work.tile([128, 2, D], BF16, tag="moeytok")
                for m in range(2):
                    yp = gp([128, CAP])
                    for k in range(8):
                        nc.tensor.matmul(out=yp[:], lhsT=w2t[:, k, m * 128:(m + 1) * 128],
                                         rhs=hT[:, k, :], start=(k == 0), stop=(k == 7))
                    ysb = work.tile([128, CAP], BF16, tag="moeysb")
                    nc.vector.tensor_scalar(out=ysb[:], in0=yp[:], scalar1=b2col[:, e, m:m + 1],
                                            scalar2=None, op0=ALU.add)
                    # transpose [128 d, 192 slots] -> token-major
                    tp = gp([128, 256], BF16)
                    nc.tensor.transpose(out=tp[:, 0:128], in_=ysb[:, 0:128], identity=ident_bf[:])
                    nc.tensor.transpose(out=tp[:64, 128:256], in_=ysb[:, 128:CAP], identity=ident_bf[:])
                    nc.vector.tensor_copy(ytok[:, 0, m * 128:(m + 1) * 128], tp[:, 0:128])
                    nc.vector.tensor_copy(ytok[:64, 1, m * 128:(m + 1) * 128], tp[:64, 128:256])
                nc.sync.dma_start(out=y_dram[e * CAP:e * CAP + 128, :], in_=ytok[:, 0, :])
                nc.sync.dma_start(out=y_dram[e * CAP + 128:(e + 1) * CAP, :], in_=ytok[:64, 1, :])
            # combine: gather the two expert outputs per token
            out_tok = work.tile([128, 4, D], F32, tag=out_tag, name=out_tag + "_o")
            for t in range(4):
                y1 = work.tile([128, D], BF16, tag="moey1")
                nc.gpsimd.indirect_dma_start(
                    out=y1[:], out_offset=None, in_=y_dram[:, :],
                    in_offset=IndirectOffsetOnAxis(ap=slot_i[2 * t][:, 0:1], axis=0))
                y2 = work.tile([128, D], BF16, tag="moey2")
                nc.gpsimd.indirect_dma_start(
                    out=y2[:], out_offset=None, in_=y_dram[:, :],
                    in_offset=IndirectOffsetOnAxis(ap=slot_i[2 * t + 1][:, 0:1], axis=0))
                t1 = small.tile([128, D], F32, tag="moec1")
                nc.vector.tensor_scalar(out=t1[:], in0=y1[:], scalar1=s1l[t][:, 0:1],
                                        scalar2=None, op0=ALU.mult)
                nc.vector.tensor_add(out=out_tok[:, t, :], in0=xtok_f32[:, t, :], in1=t1[:])
                nc.vector.tensor_scalar(out=t1[:], in0=y2[:], scalar1=s2l[t][:, 0:1],
                                        scalar2=None, op0=ALU.mult)
                nc.vector.tensor_add(out=out_tok[:, t, :], in0=out_tok[:, t, :], in1=t1[:])
            return out_tok

        # ================= forward pass =================
        # src (own batch, full sequence)
        srcT = persist.tile([128, 2, HW], F32R, tag="srcT")
        proj(wiT, bi, xT, HW, 3, out=srcT)
        srcp = work.tile([128, 2, HW], F32R, tag="kin", name="srcp_t")
        nc.vector.tensor_add(out=srcp[:], in0=srcT[:], in1=posT[:])

        # MHA#1 (layer-invariant)
        qin1 = work.tile([128, 2, HALF], F32R, tag="qinT", name="qin1_t")
        nc.vector.tensor_add(out=qin1[:], in0=leT[:], in1=posTh[:])
        qT1 = proj(wat["qs"], wat["bqs"], qin1, HALF, 2, out_tag="qT", odt=F32R)
        kT1 = proj(wat["ks"], wat["bks"], srcp, HW, 2, out_tag="bigx", odt=F32R)
        v331 = v_tokens(srcT, wat["vs"], wat["bvs"], HW)
        t2 = attn(qT1, kT1, v331, wat["os"], wat["bos"], HW, out_tag="attnO")
        u1 = work.tile([128, 2, HALF], F32, tag="uT", name="u1_t")
        nc.vector.tensor_add(out=u1[:], in0=leT[:], in1=t2[:])
        tgt_l = [ln_feat(u1, 0, "tgtA"), ln_feat(u1, 4, "tgtl1")]

        decT_full = persist.tile([128, 2, HW], F32R, tag="decT_full")
        nc.vector.tensor_copy(decT_full[:], srcT[:])

        for layer in range(NDEC):
            il = 4 * layer
            tgtT = tgt_l[layer]
            # MHA#2 (cross to dec)
            kin = work.tile([128, 2, HW], F32R, tag="kin")
            nc.vector.tensor_add(out=kin[:], in0=decT_full[:], in1=posT[:])
            qin = work.tile([128, 2, HALF], F32R, tag="qinT", name="qin_t")
            nc.vector.tensor_add(out=qin[:], in0=tgtT[:], in1=posTh[:])
            qT = proj(wat["qc"], wat["bqc"], qin, HALF, 2, out_tag="qT", odt=F32R)
            kT = proj(wat["kc"], wat["bkc"], kin, HW, 2, out_tag="bigx", odt=F32R)
            v33 = v_tokens(decT_full, wat["vc"], wat["bvc"], HW)
            a2 = attn(qT, kT, v33, wat["oc"], wat["boc"], HW, out_tag="attnO")
            u2 = work.tile([128, 2, HALF], F32, tag="uT", name="u2_t")
            nc.vector.tensor_add(out=u2[:], in0=tgtT[:], in1=a2[:])
            tgt2T = ln_feat(u2, il + 1, "tgtA")
            # MoE a
            odA = moe(tgt2T, gw1, gb1, dd["W1a"], dd["W2a"], b1sb["a"], b2sb["a"], "moeO")
            od_tok = ln_tok(odA, il + 2, "tokO")
            odT = tok_to_feat(od_tok, "featX")
            # MoE b
            odB = moe(odT, gw2, gb2, dd["W1b"], dd["W2b"], b1sb["b"], b2sb["b"], "moeO")
            dec_tok = ln_tok(odB, il + 3, "tokO")
            decTh = tok_to_feat(dec_tok, "featX", odt=F32R)
            if layer == 0:
                nc.sync.dma_start(out=_r3(cc_in), in_=decTh[:])
                nc.gpsimd.collective_compute(
                    "AllGather", ALU.bypass,
                    ins=[cc_in[:]], outs=[cc_out[:]],
                    replica_groups=[list(range(N_CORES))])
                for a in range(2):
                    for hh2 in range(2):
                        nc.gpsimd.indirect_dma_start(
                            out=decT_full[:, a, hh2 * HALF:(hh2 + 1) * HALF],
                            out_offset=None, in_=cc_out[:, :],
                            in_offset=IndirectOffsetOnAxis(
                                ap=agrows[:, 2 * a + hh2:2 * a + hh2 + 1], axis=0))
            else:
                # output projection
                woutT = work.tile([128, 2, CINP], F32R, tag="qinT", name="woutT_t")
                nc.sync.dma_start(out=woutT[:], in_=_r3(dd["WoutT"]))
                for m in range(3):
                    mc = 128 if m < 2 else CINP - 256
                    p = gp([128, HALF])
                    for k in range(2):
                        nc.tensor.matmul(out=p[:mc, :],
                                         lhsT=r32(woutT[:, k, m * 128:m * 128 + mc]),
                                         rhs=r32(decTh[:, k, :]), start=(k == 0), stop=False)
                    nc.tensor.matmul(out=p[:mc, :], lhsT=r32(bout[:, m * 128:m * 128 + mc]),
                                     rhs=r32(onesrow[:, :HALF]), start=False, stop=True)
                    osb = work.tile([128, HALF], F32, tag="outsb")
                    nc.vector.tensor_copy(osb[:mc, :], p[:mc, :])
                    nc.sync.dma_start(out=out_d[m * 128:m * 128 + mc, :], in_=osb[:mc, :])


# ---------------------------------------------------------------------------
# Host-side driver
# ---------------------------------------------------------------------------
_CACHE = {}


def _prep_maps(inputs):
    x = np.asarray(inputs["x"], np.float32)
    pos = _sine_pos()                                  # [1024, 256]
    le = np.asarray(inputs["learned_embed"], np.float32)

    def pad_rows(a, n):
        out = np.zeros((n, a.shape[1]), np.float32)
        out[:a.shape[0]] = a
        return out

    WiT = pad_rows(np.asarray(inputs["Wi"], np.float32).T, CINP)      # [384, 256]
    WoutT = np.asarray(inputs["Wout"], np.float32).T                  # [256, 272]
    WoutTp = np.zeros((D, CINP), np.float32); WoutTp[:, :CIN] = WoutT
    boutp = np.zeros((1, CINP), np.float32)
    boutp[0, :CIN] = np.asarray(inputs["bout"], np.float32)

    shared = {
        "posT": np.ascontiguousarray(pos.T),
        "WiT": np.ascontiguousarray(WiT),
        "bi": np.asarray(inputs["bi"], np.float32)[None, :],
        "ln_gf": np.asarray(inputs["ln_g"], np.float32)[[0, 1, 4, 5]].reshape(1, -1),
        "ln_bf": np.asarray(inputs["ln_b"], np.float32)[[0, 1, 4, 5]].reshape(1, -1),
        "ln_gt": np.asarray(inputs["ln_g"], np.float32)[[2, 3, 6, 7]].reshape(1, -1),
        "ln_bt": np.asarray(inputs["ln_b"], np.float32)[[2, 3, 6, 7]].reshape(1, -1),
        "gw1T": np.ascontiguousarray(np.asarray(inputs["gate_w1"], np.float32).T),
        "gb1": np.asarray(inputs["gate_b1"], np.float32)[None, :],
        "gw2T": np.ascontiguousarray(np.asarray(inputs["gate_w2"], np.float32).T),
        "gb2": np.asarray(inputs["gate_b2"], np.float32)[None, :],
        "W1a": np.asarray(inputs["W1a"]).astype(ml_dtypes.bfloat16),
        "W2a": np.asarray(inputs["W2a"]).astype(ml_dtypes.bfloat16),
        "W1b": np.asarray(inputs["W1b"]).astype(ml_dtypes.bfloat16),
        "W2b": np.asarray(inputs["W2b"]).astype(ml_dtypes.bfloat16),
        "b1a": np.asarray(inputs["b1a"], np.float32),
        "b2a": np.asarray(inputs["b2a"], np.float32),
        "b1b": np.asarray(inputs["b1b"], np.float32),
        "b2b": np.asarray(inputs["b2b"], np.float32),
        "WoutT": WoutTp,
        "bout": boutp,
        "capbase": (np.arange(E, dtype=np.float32) * CAP)[None, :],
        "onesr": np.ones((1, 512), np.float32),
    }
    for t in ("s", "c"):
        Wqkv = np.asarray(inputs[f"Wqkv_{t}"], np.float32)
        bqkv = np.asarray(inputs[f"bqkv_{t}"], np.float32)
        Wo = np.asarray(inputs[f"Wo_{t}"], np.float32)
        bo = np.asarray(inputs[f"bo_{t}"], np.float32)
        for i, w in enumerate(("q", "k", "v")):
            shared[f"W{w}{t}T"] = np.ascontiguousarray(Wqkv[i * D:(i + 1) * D].T)
            shared[f"b{w}{t}"] = bqkv[i * D:(i + 1) * D][None, :]
        shared[f"Wo{t}T"] = np.ascontiguousarray(Wo.T)
        shared[f"bo{t}"] = bo[None, :]

    in_maps = []
    for c in range(N_CORES):
        b, half = c // 2, c % 2
        m = dict(shared)
        xb = x[b].reshape(CIN, HW)
        xbp = np.zeros((CINP, HW), np.float32); xbp[:CIN] = xb
        m["xT"] = xbp
        m["posTh"] = np.ascontiguousarray(pos.T[:, half * HALF:(half + 1) * HALF])
        ag = np.zeros((128, 4), np.int32)
        for a in range(2):
            for hh2 in range(2):
                ag[:, 2 * a + hh2] = (2 * b + hh2) * D + a * 128 + np.arange(128)
        m["agrows"] = ag
        m["leT"] = np.ascontiguousarray(le.T[:, half * HALF:(half + 1) * HALF])
        in_maps.append(m)
    return in_maps


def kernel(**inputs):
    from concourse.bass_utils import run_bass_kernel_spmd
    if "nc" not in _CACHE:
        _CACHE["nc"] = build_nc()
    nc = _CACHE["nc"]
    in_maps = _prep_maps(inputs)
    res = run_bass_kernel_spmd(nc, in_maps, list(range(N_CORES)))
    # assemble output: recT per core = [CINP, 512] feature-major for (batch, half)
    out = np.zeros((B, CIN, H * W), np.float32)
    for c in range(N_CORES):
        b, half = c // 2, c % 2
        rec = res.results[c]["recT"][:CIN]          # [272, 512]
        out[b, :, half * HALF:(half + 1) * HALF] = rec
    return out.reshape(B, CIN, H, W)


if __name__ == "__main__":
    import reference as R
    inp = {k: np.asarray(v) for k, v in R.setup_inputs().items()}
    got = kernel(**inp)
    print("kernel out:", got.shape, got.dtype)
